# revision 24
# baseline (speedup 1.0000x reference)
"""Bahdanau attention kernel for Trainium2 (Bass/Tile), 8-core data-parallel.

Reference computation (per batch row b):
    Wh = W[:, :512]; We = W[:, 512:]
    h_proj = hidden @ Wh.T + b                  [B, 512]
    e_proj = enc @ We.T                         [B, S, 512]
    energy = tanh(h_proj[:, None, :] + e_proj)  [B, S, 512]
    scores = energy @ v                         [B, S]
    scores = where(mask == 0, -1e10, scores)
    out    = softmax(scores, axis=1)            [B, S]

Sharding: data-parallel over batch, 4 rows per core; W/b/v replicated.

Per-core dataflow ("transposed" orientation — d on partitions):
  - W is transposed on-chip once via PE-transpose -> WT[in, d] (WhT+WeT).
  - enc arrives as float32r via casting SWDGE DMAs; enc tiles [s,e] are
    PE-transposed in f32r (1.5 cyc/row vs fp32's 2) to encT[e, s] so the big
    matmul e_projT[d, s] = WeT.T @ encT contracts e on partitions. Matmuls
    run in float32r (~254 ns/instr measured at N=512, vs ~850 for fp32) with
    ~TF32 precision (measured rel-l2 error 6e-4 on the final softmax).
  - tanh is fused with the +h_proj+b bias via ScalarE ACT (per-partition
    bias), reading matmul PSUM directly.
  - scores are computed on PE with a "v embedding": lhsT column b holds v,
    other columns 0, so batch b's scores accumulate into PSUM row b. This
    yields scores in [batch=partition, s=free] layout for a free-dim softmax.
  - masking is one tensor_tensor add of (mask-1)*1e10; softmax is
    reduce_max(negate) -> ACT Exp with bias=-max and accum_out=sum ->
    reciprocal -> tensor_scalar mul. (A fused tensor_tensor_reduce variant
    hung the hardware and was removed.)

Loop structure: per (batch, s-chunk) iteration all 32 transposes run first
(transpose PSUM pool bufs=4 so the PSUM->SBUF copies never stall them), then
the 4 matmul accumulation groups run k-outer/ec-inner so the first MM needs
only encT[ec=0] (copied long before). The score (v-dot) matmuls of iteration
i are deferred until after iteration i+1's transposes so PE never waits on
the ACT tanh that produces their inputs. Steady state is PE-bound with no
gaps (HW-measured ~96 us per full pass per core).
"""

import numpy as np

import concourse.bass as bass  # noqa: F401
import concourse.mybir as mybir
import concourse.tile as tile
from concourse import bacc
from concourse.bass_utils import run_bass_kernel_spmd
from concourse.masks import make_identity

F32 = mybir.dt.float32
F32R = mybir.dt.float32r
I32 = mybir.dt.int32
AF = mybir.ActivationFunctionType

B, S, E2, DH = 32, 1024, 1024, 512  # batch, seq, 2*enc_hid, dec_hid
NCORES = 8
BL = B // NCORES  # 4 batch rows per core
NEG = -1e10

P = 128
KD = DH // P          # 4 d-chunks
KE = E2 // P          # 8 e-chunks
KIN = (DH + E2) // P  # 12 input-dim chunks of W
SC = 512              # s-chunk (free dim of main matmuls)
NSC = S // SC         # 2 s-chunks per batch row


def _build_kernel(reps=1):
    nc = bacc.Bacc(
        "TRN2",
        target_bir_lowering=False,
        debug=False,
        enable_asserts=False,
        num_devices=NCORES,
    )
    hid_d = nc.dram_tensor("hidden", [BL, DH], F32, kind="ExternalInput").ap()
    enc_d = nc.dram_tensor("enc", [BL, S, E2], F32, kind="ExternalInput").ap()
    mask_d = nc.dram_tensor("mask", [BL, S], I32, kind="ExternalInput").ap()
    w_d = nc.dram_tensor("W", [DH, DH + E2], F32, kind="ExternalInput").ap()
    b_d = nc.dram_tensor("b", [DH], F32, kind="ExternalInput").ap()
    v_d = nc.dram_tensor("v", [DH], F32, kind="ExternalInput").ap()
    out_d = nc.dram_tensor("out", [BL, S], F32, kind="ExternalOutput").ap()

    with tile.TileContext(nc) as tc:
        with (
            tc.tile_pool(name="const", bufs=1) as constp,
            tc.tile_pool(name="nat", bufs=3) as natp,
            tc.tile_pool(name="encT", bufs=2) as encp,
            tc.tile_pool(name="energy", bufs=2) as enp,
            tc.tile_pool(name="small", bufs=1) as smp,
            tc.tile_pool(name="pt", bufs=4, space="PSUM") as ptp,
            tc.tile_pool(name="pmm", bufs=2, space="PSUM") as mmp,
            tc.tile_pool(name="pscore", bufs=1, space="PSUM") as scp,
        ):
            # ---------------- setup ----------------
            ident = constp.tile([P, P], F32)
            make_identity(nc, ident[:])
            ident_r_t = constp.tile([P, P], F32R)
            nc.vector.tensor_copy(ident_r_t[:], ident[:])
            ident_r = ident_r_t[:]

            w_nat = constp.tile([P, KD, DH + E2], F32R)  # [p, j, c]: W[j*128+p, c]
            nc.gpsimd.dma_start(w_nat[:], w_d.rearrange("(j p) c -> p j c", p=P))

            hid_sb = constp.tile([BL, DH], F32R)
            nc.gpsimd.dma_start(hid_sb[:], hid_d)
            mask_sb = constp.tile([BL, S], I32)
            nc.sync.dma_start(mask_sb[:], mask_d)
            b_sb = constp.tile([P, KD], F32)  # b[k*128+p]
            nc.sync.dma_start(b_sb[:], b_d.rearrange("(k p) -> p k", p=P))
            v_sb = constp.tile([P, KD], F32)
            nc.sync.dma_start(v_sb[:], v_d.rearrange("(k p) -> p k", p=P))

            # WT[p, i, c] = W[c, i*128+p]  (full transpose of W, f32r)
            wT = constp.tile([P, KIN, DH], F32R)
            for i in range(KIN):
                pt = ptp.tile([P, SC], F32R, tag="pt")
                for j in range(KD):
                    nc.tensor.transpose(
                        pt[:, j * P : (j + 1) * P],
                        w_nat[:, j, i * P : (i + 1) * P],
                        ident_r,
                    )
                if i % 2:
                    nc.vector.tensor_copy(wT[:, i, :], pt[:].bitcast(F32))
                else:
                    nc.scalar.copy(wT[:, i, :], pt[:].bitcast(F32))

            # hidden transposed: hidT[p, kk, m] = hidden[m, kk*128+p]
            hidT = constp.tile([P, KD, BL], F32R)
            pt = ptp.tile([P, SC], F32R, tag="pt")
            for kk in range(KD):
                nc.tensor.transpose(
                    pt[:, kk * BL : (kk + 1) * BL],
                    hid_sb[:, kk * P : (kk + 1) * P],
                    ident_r[0:BL, 0:BL],
                )
            nc.scalar.copy(hidT[:], pt[:, : KD * BL].bitcast(F32))

            # h_proj (transposed) + b:
            # hpb[p, k, m] = sum_dec W[k*128+p, dec]*hidden[m, dec] + b[k*128+p]
            hpb = constp.tile([P, KD, BL], F32)
            pt2 = mmp.tile([P, SC], F32, tag="pm")
            for k in range(KD):
                for kk in range(KD):
                    nc.tensor.matmul(
                        pt2[:, k * BL : (k + 1) * BL],
                        wT[:, kk, k * P : (k + 1) * P],
                        hidT[:, kk, :],
                        start=(kk == 0),
                        stop=(kk == KD - 1),
                    )
            for k in range(KD):
                nc.vector.tensor_scalar_add(
                    hpb[:, k, :], pt2[:, k * BL : (k + 1) * BL], b_sb[:, k : k + 1]
                )

            # v embedding: v_emb[p, k, bi, m] = v[k*128+p] if m == bi else 0
            v_emb = constp.tile([P, KD, BL, BL], F32R)
            zf = constp.tile([P, KD, BL, BL], F32)
            nc.vector.memset(zf[:], 0.0)
            nc.vector.tensor_copy(v_emb[:], zf[:])
            for k in range(KD):
                for bi in range(BL):
                    nc.vector.tensor_copy(
                        v_emb[:, k, bi, bi : bi + 1], v_sb[:, k : k + 1]
                    )

            # additive mask: (mask-1)*1e10 -> 0 where mask==1, -1e10 where mask==0
            maskneg = constp.tile([BL, S], F32)
            nc.scalar.activation(
                maskneg[:], mask_sb[:], AF.Copy, bias=-1e10, scale=1e10
            )

            # scores PSUM accumulator [batch, sc, s]
            scores_ps = scp.tile([BL, NSC, SC], F32)

            # ---------------- main loop ----------------
            # reps>1 repeats the identical computation for slope-based HW
            # timing (output unchanged: scores accumulation restarts per rep).
            # The score (v-dot) matmuls of iteration i are deferred until
            # after iteration i+1's transposes so PE never stalls on the
            # ACT tanh that produces their inputs.
            pending = []  # deferred vdot matmuls: list of (bi, sc, k, energy)

            def flush_vdots():
                for pbi, psc, pk, pen in pending:
                    nc.tensor.matmul(
                        scores_ps[:, psc, :],
                        v_emb[:, pk, pbi, :],
                        pen[:, pk, :],
                        start=(pbi == 0 and pk == 0),
                        stop=(pbi == BL - 1 and pk == KD - 1),
                    )
                pending.clear()

            for _rep in range(reps):
                for bi in range(BL):
                    for sc in range(NSC):
                        nat = natp.tile([P, SC // P, E2], F32R)
                        nc.gpsimd.dma_start(
                            nat[:],
                            enc_d[bi, sc * SC : (sc + 1) * SC, :].rearrange(
                                "(ss p) e -> p ss e", p=P
                            ),
                        )
                        encT = encp.tile([P, KE, SC], F32R)
                        for ec in range(KE):
                            pt = ptp.tile([P, SC], F32R, tag="pt")
                            for ss in range(SC // P):
                                nc.tensor.transpose(
                                    pt[:, ss * P : (ss + 1) * P],
                                    nat[:, ss, ec * P : (ec + 1) * P],
                                    ident_r,
                                )
                            if ec % 2:
                                nc.vector.tensor_copy(
                                    encT[:, ec, :], pt[:].bitcast(F32)
                                )
                            else:
                                nc.scalar.copy(encT[:, ec, :], pt[:].bitcast(F32))
                        flush_vdots()
                        energy = enp.tile([P, KD, SC], F32R)
                        for k in range(KD):
                            pm = mmp.tile([P, SC], F32, tag="pm")
                            for ec in range(KE):
                                nc.tensor.matmul(
                                    pm[:],
                                    wT[:, KD + ec, k * P : (k + 1) * P],
                                    encT[:, ec, :],
                                    start=(ec == 0),
                                    stop=(ec == KE - 1),
                                )
                            nc.scalar.activation(
                                energy[:, k, :],
                                pm[:],
                                AF.Tanh,
                                bias=hpb[:, k, bi : bi + 1],
                            )
                            pending.append((bi, sc, k, energy))
            flush_vdots()

            # ---------------- masked softmax over s ----------------
            sm = smp.tile([BL, S], F32)
            for sc in range(NSC):
                nc.vector.tensor_add(
                    sm[:, sc * SC : (sc + 1) * SC],
                    scores_ps[:, sc, :],
                    maskneg[:, sc * SC : (sc + 1) * SC],
                )
            negmax = smp.tile([BL, 1], F32)
            nc.vector.tensor_reduce(
                negmax[:], sm[:], axis=mybir.AxisListType.X,
                op=mybir.AluOpType.max, negate=True,
            )
            expv = smp.tile([BL, S], F32)
            sumexp = smp.tile([BL, 1], F32)
            nc.scalar.activation(
                expv[:], sm[:], AF.Exp, bias=negmax[:], accum_out=sumexp[:]
            )
            rec = smp.tile([BL, 1], F32)
            nc.vector.reciprocal(rec[:], sumexp[:])
            outsb = smp.tile([BL, S], F32)
            nc.vector.tensor_scalar_mul(outsb[:], expv[:], rec[:])
            nc.sync.dma_start(out_d, outsb[:])

    nc.compile()
    return nc


_NC_CACHE = None
LAST_RESULTS = None


def kernel(hidden, encoder_outputs, mask, W, b, v, _trace=False):
    global _NC_CACHE, LAST_RESULTS
    if _NC_CACHE is None:
        _NC_CACHE = _build_kernel()
    nc = _NC_CACHE

    hidden = np.ascontiguousarray(np.asarray(hidden, dtype=np.float32))
    enc = np.ascontiguousarray(np.asarray(encoder_outputs, dtype=np.float32))
    mask = np.ascontiguousarray(np.asarray(mask, dtype=np.int32))
    W = np.ascontiguousarray(np.asarray(W, dtype=np.float32))
    b = np.ascontiguousarray(np.asarray(b, dtype=np.float32))
    v = np.ascontiguousarray(np.asarray(v, dtype=np.float32))

    in_maps = []
    for c in range(NCORES):
        sl = slice(c * BL, (c + 1) * BL)
        in_maps.append(
            {
                "hidden": np.ascontiguousarray(hidden[sl]),
                "enc": np.ascontiguousarray(enc[sl]),
                "mask": np.ascontiguousarray(mask[sl]),
                "W": W,
                "b": b,
                "v": v,
            }
        )

    res = run_bass_kernel_spmd(
        nc, in_maps, core_ids=list(range(NCORES)), trace=_trace
    )
    LAST_RESULTS = res
    return np.concatenate([r["out"] for r in res.results], axis=0)


def bench(in_maps=None, iters=30, inputs=None, reps=1, nc=None):
    """Time repeated executions with device-resident inputs (amortizes the
    axon transfer/dispatch overhead). Returns (sec/iter, core0 output).

    iters > 0: async pipelined loop (block once at the end).
    iters < 0: -iters fully-blocking trials, return the min.
    """
    import time

    import jax
    import numpy as np_
    from jax.experimental.shard_map import shard_map
    from jax.sharding import Mesh, NamedSharding, PartitionSpec

    import concourse.mybir as mybir
    from concourse.bass2jax import (
        _bass_exec_p,
        install_neuronx_cc_hook,
        partition_id_tensor,
    )

    global _NC_CACHE
    if nc is None:
        if reps == 1:
            if _NC_CACHE is None:
                _NC_CACHE = _build_kernel()
            nc = _NC_CACHE
        else:
            nc = _build_kernel(reps)
    install_neuronx_cc_hook()

    if in_maps is None:
        assert inputs is not None
        hidden = np_.asarray(inputs["hidden"], dtype=np_.float32)
        enc = np_.asarray(inputs["encoder_outputs"], dtype=np_.float32)
        mask = np_.asarray(inputs["mask"], dtype=np_.int32)
        W = np_.asarray(inputs["W"], dtype=np_.float32)
        b = np_.asarray(inputs["b"], dtype=np_.float32)
        v = np_.asarray(inputs["v"], dtype=np_.float32)
        in_maps = []
        for c in range(NCORES):
            sl = slice(c * BL, (c + 1) * BL)
            in_maps.append({"hidden": hidden[sl], "enc": enc[sl], "mask": mask[sl],
                            "W": W, "b": b, "v": v})

    partition_name = nc.partition_id_tensor.name if nc.partition_id_tensor else None
    in_names, out_names, out_avals, zero_outs = [], [], [], []
    for alloc in nc.m.functions[0].allocations:
        if not isinstance(alloc, mybir.MemoryLocationSet):
            continue
        name = alloc.memorylocations[0].name
        if alloc.kind == "ExternalInput":
            if name != partition_name:
                in_names.append(name)
        elif alloc.kind == "ExternalOutput":
            shape = tuple(alloc.tensor_shape)
            dtype = mybir.dt.np(alloc.dtype)
            out_names.append(name)
            out_avals.append(jax.core.ShapedArray(shape, dtype))
            zero_outs.append(np_.zeros(shape, dtype))
    n_params = len(in_names)
    n_outs = len(out_avals)
    in_names.extend(out_names)
    if partition_name is not None:
        in_names.append(partition_name)

    def _body(*args):
        operands = list(args)
        if partition_name is not None:
            operands.append(partition_id_tensor())
        outs = _bass_exec_p.bind(
            *operands,
            out_avals=tuple(out_avals),
            in_names=tuple(in_names),
            out_names=tuple(out_names),
            lowering_input_output_aliases=(),
            sim_require_finite=True,
            sim_require_nnan=True,
            nc=nc,
        )
        return tuple(outs)

    devices = jax.devices()[:NCORES]
    mesh = Mesh(np_.asarray(devices), ("core",))
    in_specs = (PartitionSpec("core"),) * (n_params + n_outs)
    out_specs = (PartitionSpec("core"),) * n_outs
    # no donation so device inputs survive across iterations
    sharded = jax.jit(
        shard_map(_body, mesh=mesh, in_specs=in_specs, out_specs=out_specs,
                  check_rep=False),
        keep_unused=True,
    )
    shard = NamedSharding(mesh, PartitionSpec("core"))
    concat_in = [
        jax.device_put(
            np_.concatenate([np_.asarray(in_maps[c][nm]) for c in range(NCORES)],
                            axis=0),
            shard,
        )
        for nm in in_names[:n_params]
    ]
    concat_zeros = [
        jax.device_put(np_.zeros((NCORES * z.shape[0], *z.shape[1:]), z.dtype), shard)
        for z in zero_outs
    ]
    # warmup + correctness reference output
    outs = sharded(*concat_in, *concat_zeros)
    jax.block_until_ready(outs)
    if iters < 0:
        best = None
        for _ in range(-iters):
            t0 = time.time()
            outs = sharded(*concat_in, *concat_zeros)
            jax.block_until_ready(outs)
            dt = time.time() - t0
            best = dt if best is None else min(best, dt)
        return best, np_.asarray(outs[0])
    t0 = time.time()
    for _ in range(iters):
        outs = sharded(*concat_in, *concat_zeros)
    jax.block_until_ready(outs)
    dt = (time.time() - t0) / iters
    out_np = np_.asarray(outs[0])
    return dt, out_np
